# revision 29
# baseline (speedup 1.0000x reference)
"""Single-head causal attention (B=256, T=256, C=1024, D=64) on 8 TRN2 NeuronCores.

Data-parallel over batch (32 batches/core). v2 scheme halves the x DMA
traffic vs the fp8 hi/lo-pair baseline:

  * x ships as fp8-e4m3 xh for ALL positions plus the xl residual for only
    the first 64 sequence positions (5120B/partition/superbatch, one DMA).
    Early positions dominate both signal and error of causal attention
    (softmax over few values), so correcting v rows s<64 and q/k rows t<64
    recovers most of the accuracy of a full hi/lo pair at 56% of the bytes:
        q|k^T [128,T]: xh@(wh+wl)  + xl@wh on t<64      (rel err 1.06e-2
        v     [T,D]:   xh@(wvh+wvl) + xl@wvh on s<64     vs the 2e-2 gate)
    Weights are host-prescaled by 8 so their fp8 hi/lo splits stay normal;
    the 8x comes out in the exp scale and the fused denominator column.
  * Per-superbatch (2 batches): DMA 1820ns (load) + 182ns (store) vs PE
    ~1975ns -- DMA and PE are both ~98% busy (ridge). The load stream is
    gapless from ~5.5us to ~34.3us. Engine split tuned so no dependency
    cycle exceeds the 2002ns DMA cadence:
      - PE program order per iteration: qk(sb) -> scores(sb-1) -> v(sb)
        -> finale(sb-3). Scores sit right after qk so the exp->mask chain
        (ACT+Pool serial ~1.9us) finishes an iteration before finale
        needs it (finale lag 3).
      - DVE: q copy + k copy + reciprocal (both copies on DVE keeps ACT's
        serial [exp b0, exp b1] chain out of the scores->scores cycle).
      - ACT: exps + v copy + finale o2->SBUF copy; Pool: causal masks
        (affine_select) + denominator memsets + finale multiply.
  * Head: wqk ships as separate hi/lo DMAs and sb0 loads per batch
    (batch-major copy) so the first projections start ~1.3us in. Tail:
    sb15 loads per batch into per-batch PSUM tiles/copies, its scores
    run at the drain head, fins 13/14 get separate PSUM banks with
    direct-DVE normalize, and drain stores ride separate DGE queues
    (SP/ACT) so none parks behind an earlier store's data.
"""

import numpy as np
import ml_dtypes

import concourse.bacc as bacc
import concourse.mybir as mybir
import concourse.tile as tile
from concourse.bass_utils import run_bass_kernel_spmd

B, T, C, D = 256, 256, 1024, 64
NCORES = 8
BPC = B // NCORES  # batches per core
NSB = BPC // 2  # superbatches (2 batches each)
CCH = C // 128  # contraction chunks
NCP = CCH // 2  # chunk pairs (DoubleRow contracts 2 chunks/instruction)
S0 = 64  # xl residual shipped for s < S0 only
WS = 8.0  # host weight prescale
SCALE = float(C) ** -0.5

BF16 = mybir.dt.bfloat16
F32 = mybir.dt.float32
F8 = mybir.dt.float8e4
E4M3 = ml_dtypes.float8_e4m3
DR = mybir.MatmulPerfMode.DoubleRow

TRACE = False
LAST_RESULT = None


def _build(pf=3):
    nc = bacc.Bacc(
        "TRN2", target_bir_lowering=False, debug=False, num_devices=NCORES
    )
    # x: [sb, partition, row, batch, quarter, 64] -- rows 0..7 hold xh chunk r
    # as [b, t(=4*64)], rows 8..9 hold xl (s<64) chunk 4*(r-8)+q as [b, q, s].
    xt_d = nc.dram_tensor("xt_d", [NSB, 128, 10, 2, 4, 64], F8, kind="ExternalInput")
    # sb0 and sb15 duplicated batch-major for per-batch head/tail loads
    xe_d = nc.dram_tensor("xe_d", [2, 2, 128, 10, 4, 64], F8, kind="ExternalInput")
    wqk_d = nc.dram_tensor("wqk_d", [2, 128, CCH, 128], F8, kind="ExternalInput")
    wv_d = nc.dram_tensor("wv_d", [128, CCH, 2, 64], F8, kind="ExternalInput")
    out = nc.dram_tensor("out", [NSB, 128, 2, 2, D], BF16, kind="ExternalOutput")

    with tile.TileContext(nc) as tc:
        with (
            tc.tile_pool(name="singles", bufs=1) as singles,
            tc.tile_pool(name="xp", bufs=pf + 1) as xp,
            tc.tile_pool(name="sbp", bufs=5) as sbp,
            tc.tile_pool(name="ep", bufs=9) as ep,
            tc.tile_pool(name="vp", bufs=6) as vp,
            tc.tile_pool(name="stp", bufs=5) as stp,
            tc.tile_pool(name="rp", bufs=3) as rp,
            tc.tile_pool(name="ocp", bufs=2) as ocp,
            tc.tile_pool(name="qk_ps", bufs=2, space="PSUM") as qk_psp,
            tc.tile_pool(name="sc_ps", bufs=3, space="PSUM") as sc_psp,
            tc.tile_pool(name="v_ps", bufs=2, space="PSUM") as v_psp,
            tc.tile_pool(name="o_ps", bufs=1, space="PSUM") as o_psp,
        ):
            # weight loads split so the first projection group (hi terms of
            # batch 0) can start as early as possible
            wqk_sb = singles.tile([128, 2, CCH, 128], F8)
            nc.sync.dma_start(wqk_sb[:, 0], wqk_d[0])
            # sb0 per-batch tiles (batch-major source)
            t0b = [singles.tile([128, 10, 4, 64], F8, name=f"t0b{b}") for b in range(2)]
            nc.sync.dma_start(t0b[0], xe_d[0, 0])
            nc.sync.dma_start(wqk_sb[:, 1], wqk_d[1])
            wv_sb = singles.tile([128, CCH, 2, 64], F8)
            nc.sync.dma_start(wv_sb, wv_d[:])
            nc.sync.dma_start(t0b[1], xe_d[0, 1])

            # causal triangle for DVE-side masking at the tail
            tri = singles.tile([128, 128], BF16)
            nc.gpsimd.memset(tri, 1.0)
            nc.gpsimd.affine_select(
                out=tri, in_=tri,
                compare_op=mybir.AluOpType.is_ge,
                fill=0.0, base=0, pattern=[[1, 128]], channel_multiplier=-1,
            )

            xt_tiles = {}
            t15b = [None, None]

            def load_sb(k):
                if k == NSB - 1:
                    t15b[0] = singles.tile([128, 10, 4, 64], F8, name="t15b0")
                    t15b[1] = singles.tile([128, 10, 4, 64], F8, name="t15b1")
                    nc.sync.dma_start(t15b[0], xe_d[1, 0])
                    nc.sync.dma_start(t15b[1], xe_d[1, 1])
                else:
                    t = xp.tile([128, 10, 2, 4, 64], F8, tag="xt")
                    nc.sync.dma_start(t, xt_d[k])
                    xt_tiles[k] = t

            # --- access helpers -------------------------------------------
            def xh_qk_rhs(sb, cp, b=None):
                """moving xh for the q|k projection; dim1 = chunk pair."""
                if sb in (0, NSB - 1):
                    tb = (t0b if sb == 0 else t15b)[b]
                    return tb[:, 2 * cp : 2 * cp + 2, :, :]
                return xt_tiles[sb][:, 2 * cp : 2 * cp + 2, :, :, :]

            def xl_ap(sb, cp, b):
                """xl chunk pair (2cp, 2cp+1) for batch b: [128, 2, 64]."""
                r, q = 8 + cp // 2, (2 * cp) % 4
                if sb in (0, NSB - 1):
                    tb = (t0b if sb == 0 else t15b)[b]
                    return tb[:, r, q : q + 2, :]
                return xt_tiles[sb][:, r, b, q : q + 2, :]

            def xh_v_lhsT(sb, cp, b, st):
                if sb in (0, NSB - 1):
                    tb = (t0b if sb == 0 else t15b)[b]
                    return tb[:, 2 * cp : 2 * cp + 2, 2 * st : 2 * st + 2, :]
                return xt_tiles[sb][:, 2 * cp : 2 * cp + 2, b, 2 * st : 2 * st + 2, :]

            # --- stages ---------------------------------------------------
            def emit_qk(sb):
                """q|k^T projections -> PSUM (q rows 0:64, k rows 64:128).
                sb 0 runs per-batch groups in one tile; sb 15 gets two
                per-batch TILES so batch 0's drain chain starts early."""
                if sb == NSB - 1:
                    tiles = []
                    for b in range(2):
                        qk_b = qk_psp.tile([128, T], F32, tag="qk", name="qk_b")
                        i, n = 0, 2 * NCP + NCP
                        for h in range(2):
                            for cp in range(NCP):
                                nc.tensor.matmul(
                                    qk_b,
                                    lhsT=wqk_sb[:, h, 2 * cp : 2 * cp + 2, :],
                                    rhs=xh_qk_rhs(sb, cp, b),
                                    start=(i == 0), stop=False, perf_mode=DR,
                                )
                                i += 1
                        for cp in range(NCP):
                            nc.tensor.matmul(
                                qk_b[:, 0:S0],
                                lhsT=wqk_sb[:, 0, 2 * cp : 2 * cp + 2, :],
                                rhs=xl_ap(sb, cp, b),
                                start=False, stop=(i == n - 1), perf_mode=DR,
                            )
                            i += 1
                        tiles.append(qk_b)
                    return tiles
                qk_ps = qk_psp.tile([128, 2, T], F32, tag="qk")
                if sb == 0:
                    for b in range(2):
                        i, n = 0, 2 * NCP + NCP
                        for h in range(2):
                            for cp in range(NCP):
                                nc.tensor.matmul(
                                    qk_ps[:, b],
                                    lhsT=wqk_sb[:, h, 2 * cp : 2 * cp + 2, :],
                                    rhs=xh_qk_rhs(sb, cp, b),
                                    start=(i == 0), stop=False, perf_mode=DR,
                                )
                                i += 1
                        for cp in range(NCP):
                            nc.tensor.matmul(
                                qk_ps[:, b, 0:S0],
                                lhsT=wqk_sb[:, 0, 2 * cp : 2 * cp + 2, :],
                                rhs=xl_ap(sb, cp, b),
                                start=False, stop=(i == n - 1), perf_mode=DR,
                            )
                            i += 1
                else:
                    i, n = 0, 2 * NCP + 2 * NCP
                    for h in range(2):
                        for cp in range(NCP):
                            nc.tensor.matmul(
                                qk_ps[:, :, :],
                                lhsT=wqk_sb[:, h, 2 * cp : 2 * cp + 2, :],
                                rhs=xh_qk_rhs(sb, cp),
                                start=(i == 0), stop=False, perf_mode=DR,
                            )
                            i += 1
                    for cp in range(NCP):
                        for b in range(2):
                            nc.tensor.matmul(
                                qk_ps[:, b, 0:S0],
                                lhsT=wqk_sb[:, 0, 2 * cp : 2 * cp + 2, :],
                                rhs=xl_ap(sb, cp, b),
                                start=False, stop=(i == n - 1), perf_mode=DR,
                            )
                            i += 1
                return qk_ps

            def emit_v(sb):
                """v -> bf16 [128, 2, 2, D+1] with the fused denominator col."""
                v_sb = vp.tile([128, 2, 2, D + 1], BF16, tag="v")
                v_ps = v_psp.tile([128, 2, 2, D], F32, tag="v_ps")
                for b in range(2):
                    for st in range(2):
                        i = 0
                        n = 2 * NCP + (NCP if st == 0 else 0)
                        for h in range(2):
                            for cp in range(NCP):
                                nc.tensor.matmul(
                                    v_ps[:, b, st],
                                    lhsT=xh_v_lhsT(sb, cp, b, st),
                                    rhs=wv_sb[:, 2 * cp : 2 * cp + 2, h, :],
                                    start=(i == 0), stop=(i == n - 1),
                                    perf_mode=DR,
                                )
                                i += 1
                        if st == 0:
                            for cp in range(NCP):
                                nc.tensor.matmul(
                                    v_ps[0:S0, b, 0, :],
                                    lhsT=xl_ap(sb, cp, b),
                                    rhs=wv_sb[:, 2 * cp : 2 * cp + 2, 0, :],
                                    start=False, stop=(i == n - 1),
                                    perf_mode=DR,
                                )
                                i += 1
                nc.scalar.copy(v_sb[:, :, :, 0:D], v_ps)
                nc.gpsimd.memset(v_sb[:, :, :, D : D + 1], WS)
                return v_sb

            def scores_stage(qa, ka, split_mask=False):
                """scores^T + exp + causal mask for one batch (qa/ka are
                [64, 256] APs). Packed [128, 384]: cols 0:256 = (s<128,
                all t), 256:384 = (s>=128, t>=128)."""
                sc_ps = sc_psp.tile([128, 3 * 128], F32, tag="sc")
                nc.tensor.matmul(
                    sc_ps[:, 0:T], lhsT=ka[:, 0:128], rhs=qa,
                    start=True, stop=True,
                )
                nc.tensor.matmul(
                    sc_ps[:, T : T + 128], lhsT=ka[:, 128:T], rhs=qa[:, 128:T],
                    start=True, stop=True,
                )
                expT = ep.tile([128, 3 * 128], BF16, tag="expT")
                nc.scalar.activation(
                    expT, sc_ps,
                    func=mybir.ActivationFunctionType.Exp,
                    scale=SCALE / (WS * WS),
                )
                for qi, quad in enumerate((0, 256)):
                    if split_mask and qi == 0:
                        # tail: run the two quadrant masks on different
                        # engines so the finale's mm chain unblocks sooner
                        nc.vector.tensor_tensor(
                            expT[:, quad : quad + 128],
                            expT[:, quad : quad + 128],
                            tri, mybir.AluOpType.mult,
                        )
                        continue
                    nc.gpsimd.affine_select(
                        out=expT[:, quad : quad + 128],
                        in_=expT[:, quad : quad + 128],
                        compare_op=mybir.AluOpType.is_ge,
                        fill=0.0, base=0, pattern=[[1, 128]],
                        channel_multiplier=-1,
                    )
                return expT

            def final_mms(o2, bi, expT, v_sb):
                nc.tensor.matmul(
                    o2[:, bi, 0], lhsT=expT[:, 0:128], rhs=v_sb[:, bi, 0],
                    start=True, stop=True,
                )
                nc.tensor.matmul(
                    o2[:, bi, 1], lhsT=expT[:, 128:256], rhs=v_sb[:, bi, 0],
                    start=True, stop=False,
                )
                nc.tensor.matmul(
                    o2[:, bi, 1], lhsT=expT[:, 256:384], rhs=v_sb[:, bi, 1],
                    start=False, stop=True,
                )

            stages = {}

            def final_stage(sb, expTs, v_sb, pool=None, ptag="o_ps",
                            dve_norm=False, tail=False, stage=None):
                """o' matmuls + softmax normalization, both batches fused.
                Steady state spreads the normalize chain ACT (PSUM->SBUF
                copy) -> DVE (recip) -> Pool (multiply) to keep DVE under
                the DMA cadence. At drain (pool != None) ACT/Pool are the
                scarce engines, so recip+mult read PSUM directly on DVE.
                """
                drain = dve_norm or pool is not None
                o2 = (pool or o_psp).tile(
                    [128, 2, 2, D + 1], F32, tag=ptag, name="o2"
                )
                for bi in range(2):
                    final_mms(o2, bi, expTs[bi], v_sb)
                stages[sb] = stage if stage is not None else stp.tile(
                    [128, 2, 2, D], BF16, tag="stage", name="stage"
                )
                if drain:
                    recip = rp.tile([128, 2, 2], F32, tag="recip")
                    nc.vector.reciprocal(recip, o2[:, :, :, D])
                    nc.vector.tensor_tensor(
                        stages[sb],
                        o2[:, :, :, 0:D],
                        recip[:, :, :, None].to_broadcast((128, 2, 2, D)),
                        mybir.AluOpType.mult,
                    )
                    return
                o2c = ocp.tile([128, 2, 2, D + 1], F32, tag="o2c")
                nc.scalar.copy(o2c, o2)
                recip = rp.tile([128, 2, 2], F32, tag="recip")
                nc.vector.reciprocal(recip, o2c[:, :, :, D])
                # tail fins multiply on DVE (from SBUF): Pool is busy with
                # the sb14/15 mask chains there
                eng = nc.vector if tail else nc.gpsimd
                eng.tensor_tensor(
                    stages[sb],
                    o2c[:, :, :, 0:D],
                    recip[:, :, :, None].to_broadcast((128, 2, 2, D)),
                    mybir.AluOpType.mult,
                )

            # --- head loads ----------------------------------------------
            for k in range(1, min(pf, NSB)):
                load_sb(k)

            pend_sc = None  # (sb, q_sb, k_sb, v_sb) awaiting scores/exp/mask
            fin_q = []  # (sb, [expT_b0, expT_b1], v_sb) awaiting finale
            for sb in range(NSB):
                if sb + pf < NSB:
                    load_sb(sb + pf)
                if sb >= 4:
                    nc.sync.dma_start(out[sb - 4], stages.pop(sb - 4))

                # last iteration: scores(14) must precede the load-gated
                # qk(15) in the in-order PE stream, else its exp/mask chain
                # (and everything behind it) waits for the final load
                if sb == NSB - 1 and pend_sc is not None:
                    psb, pq, pk, pv = pend_sc
                    fin_q.append(
                        (psb, [scores_stage(pq[bi], pk[bi]) for bi in range(2)], pv)
                    )
                    pend_sc = None
                qk_ps = emit_qk(sb)
                if sb == NSB - 1:
                    q_aps, k_aps = [], []
                    for b in range(2):
                        q_b = sbp.tile([64, T], BF16, tag="q_b", name="q_b")
                        k_b = sbp.tile([64, T], BF16, tag="k_b", name="k_b")
                        nc.vector.tensor_copy(q_b, qk_ps[b][0:64])
                        nc.vector.tensor_copy(k_b, qk_ps[b][64:128])
                        q_aps.append(q_b[:])
                        k_aps.append(k_b[:])
                else:
                    q_sb = sbp.tile([64, 2, T], BF16, tag="q_sb")
                    k_sb = sbp.tile([64, 2, T], BF16, tag="k_sb")
                    nc.vector.tensor_copy(q_sb, qk_ps[0:64])
                    nc.vector.tensor_copy(k_sb, qk_ps[64:128])
                    q_aps = [q_sb[:, 0], q_sb[:, 1]]
                    k_aps = [k_sb[:, 0], k_sb[:, 1]]
                # scores(sb-1) right after the qk group in the PE program:
                # its exp->mask chain (ACT/Pool serial, ~1.9us) must complete
                # before finale(sb-1) runs a later iteration's PE stream.
                if pend_sc is not None:
                    psb, pq, pk, pv = pend_sc
                    fin_q.append(
                        (psb, [scores_stage(pq[bi], pk[bi]) for bi in range(2)], pv)
                    )
                    pend_sc = None
                v_sb = emit_v(sb)
                # finale lag 3: expT(sb-3) masks are a full extra iteration
                # old, so the in-order PE stream never stalls on them; the
                # last pop (fin 12) normalizes directly on DVE so its store
                # heads the drain queue without an ACT/Pool round-trip
                if len(fin_q) >= 3:
                    final_stage(*fin_q.pop(0), dve_norm=(sb == NSB - 1))
                pend_sc = (sb, q_aps, k_aps, v_sb)

            # --- drain ----------------------------------------------------
            # sb15 scores/exp/mask per batch first (the critical tail), then
            # the two pending finales on their own PSUM banks so their
            # normalize chains overlap, then sb15's per-batch finale+store.
            psb, pq, pk, v15 = pend_sc  # sb 15
            e15 = [scores_stage(pq[bi], pk[bi]) for bi in range(2)]
            nc.sync.dma_start(out[NSB - 4], stages.pop(NSB - 4))
            final_stage(*fin_q.pop(0), pool=o_psp)  # sb 13
            final_stage(*fin_q.pop(0), pool=v_psp, ptag="v_ps")  # sb 14
            # drain stores ride separate DGE queues so none waits behind an
            # earlier store whose data is still in flight
            nc.scalar.dma_start(out[NSB - 3], stages.pop(NSB - 3))
            nc.sync.dma_start(out[NSB - 2], stages.pop(NSB - 2))
            # sb15: per-batch finale -> recip -> mult -> store, normalize on
            # DVE (ACT/Pool are busy with the sb15 exp/mask chain)
            st15 = stp.tile([128, 2, 2, D], BF16, tag="stage")
            for bi in range(2):
                o2b = (sc_psp if bi == 0 else qk_psp).tile(
                    [128, 2, D + 1], F32, tag="sc" if bi == 0 else "qk",
                    name="o2b",
                )
                nc.tensor.matmul(
                    o2b[:, 0], lhsT=e15[bi][:, 0:128], rhs=v15[:, bi, 0],
                    start=True, stop=True,
                )
                nc.tensor.matmul(
                    o2b[:, 1], lhsT=e15[bi][:, 128:256], rhs=v15[:, bi, 0],
                    start=True, stop=False,
                )
                nc.tensor.matmul(
                    o2b[:, 1], lhsT=e15[bi][:, 256:384], rhs=v15[:, bi, 1],
                    start=False, stop=True,
                )
                recip = rp.tile([128, 2], F32, tag="recip_b")
                nc.vector.reciprocal(recip, o2b[:, :, D])
                nc.vector.tensor_tensor(
                    st15[:, bi],
                    o2b[:, :, 0:D],
                    recip[:, :, None].to_broadcast((128, 2, D)),
                    mybir.AluOpType.mult,
                )
                (nc.sync if bi == 0 else nc.scalar).dma_start(
                    out[NSB - 1][:, bi], st15[:, bi]
                )
    nc.compile()
    return nc


def _pack_inputs(x, Wq, Wk, Wv):
    """Host-side layout/dtype prep."""
    xt = np.ascontiguousarray(np.transpose(x, (0, 2, 1)))  # [B, C, T] f32
    xh = xt.astype(E4M3)
    xl = (xt[:, :, :S0] - xh[:, :, :S0].astype(np.float32)).astype(E4M3)

    # [B, 128, 10, 4, 64] batch-major packed array
    arr = np.empty((B, 128, 10, 4, 64), dtype=E4M3)
    arr[:, :, 0:8] = xh.reshape(B, 8, 128, 4, 64).transpose(0, 2, 1, 3, 4)
    arr[:, :, 8:10] = xl.reshape(B, 2, 4, 128, 64).transpose(0, 3, 1, 2, 4)

    def pack_w(W, m):
        w8 = W * WS
        wh = w8.astype(E4M3)
        wl = (w8 - wh.astype(np.float32)).astype(E4M3)
        return np.ascontiguousarray(
            np.stack(
                [wh.reshape(CCH, 128, m), wl.reshape(CCH, 128, m)], axis=2
            ).transpose(1, 0, 2, 3)
        )

    wqk = pack_w(np.concatenate([Wq, Wk], axis=1), 128)
    wqk = np.ascontiguousarray(wqk.transpose(2, 0, 1, 3))  # [2, 128, CCH, 128]
    wv = pack_w(Wv, D)  # [128, CCH, 2, 64]
    return arr, wqk, wv


def kernel(x: np.ndarray, Wq: np.ndarray, Wk: np.ndarray, Wv: np.ndarray) -> np.ndarray:
    global LAST_RESULT
    x = np.asarray(x, dtype=np.float32)
    Wq = np.asarray(Wq, dtype=np.float32)
    Wk = np.asarray(Wk, dtype=np.float32)
    Wv = np.asarray(Wv, dtype=np.float32)

    arr, wqk, wv = _pack_inputs(x, Wq, Wk, Wv)

    nc = _build()
    in_maps = []
    for i in range(NCORES):
        a = arr[i * BPC : (i + 1) * BPC]  # [32, 128, 10, 4, 64]
        # [NSB, 128, 10, 2, 4, 64]: batch inside row
        xt = np.ascontiguousarray(
            a.reshape(NSB, 2, 128, 10, 4, 64).transpose(0, 2, 3, 1, 4, 5)
        )
        xe = np.ascontiguousarray(
            np.stack([a[0:2], a[2 * NSB - 2 : 2 * NSB]], axis=0)
        )  # [2, 2, 128, 10, 4, 64]
        in_maps.append({"xt_d": xt, "xe_d": xe, "wqk_d": wqk, "wv_d": wv})
    res = run_bass_kernel_spmd(
        nc, in_maps, core_ids=list(range(NCORES)), trace=TRACE
    )
    LAST_RESULT = res
    # [NSB, 128, 2, 2, D] -> [NSB, 2, 2, 128, D] -> [BPC, T, D]
    outs = [
        np.ascontiguousarray(r["out"].transpose(0, 2, 3, 1, 4))
        .reshape(BPC, T, D)
        .astype(np.float32)
        for r in res.results
    ]
    return np.concatenate(outs, axis=0)


if __name__ == "__main__":
    x = np.random.randn(B, T, C).astype(np.float32)
    Wq = np.random.randn(C, D).astype(np.float32) * (C**-0.5)
    Wk = np.random.randn(C, D).astype(np.float32) * (C**-0.5)
    Wv = np.random.randn(C, D).astype(np.float32) * (C**-0.5)
    o = kernel(x, Wq, Wk, Wv)
    print(o.shape, o.dtype)
